# revision 2
# baseline (speedup 1.0000x reference)
"""BENDR contrastive-loss kernel for Trainium2 (8 NeuronCores) — v2.

Reference: for each (b, t):
  logits[b*T+t, 0]   = cos(z[b,:,t], c[b,:,t+1]) / TEMP
  logits[b*T+t, 1+k] = cos(z[b,:,t], z[b,:,n(b,t,k)]) / TEMP
with n(b,t,k) = negative_inds[b, t*K+k] (row-local), TEMP = 0.5.

Strategy (data-parallel over batch, 2 rows/core):
  - Host pre-normalizes columns: zs = z * sqrt(2)/||z_t||, cs = c[:,:,1:] *
    sqrt(2)/||c_t||, cast to bf16.  Then every logit is exactly one entry of
    the Gram-like products  zs^T zs  (negatives, and the n==t self-pair) and
    diag(zs^T cs) (positives), all in [-2, 2].
  - The Gram matrix is symmetric, so the device computes only the 16
    upper-triangle row strips per batch row:  strip tau = [128 t x (128 c-sims
    | z-sims for u in [128*tau, 2048))].  The c-sims block shares the strip's
    stationary operand, so it costs no extra LDWEIGHTS.
  - PSUM fp32 -> int8 (x63, folded into the host pre-scale) -> DMA.  int8
    round-to-nearest gives |err| <= 0.5/63 = 7.9e-3 absolute on logits whose
    absmax is 2.0, i.e. ~4e-3 relative -- well under the 2e-2 gate.
  - Host does the final pure-indexing gather (pick (t, n(t,k)) entries, using
    symmetry for u < 128*tau) and the int8 decode.
"""

import sys

for _p in ("/opt/trn_rl_repo",):
    if _p not in sys.path:
        sys.path.append(_p)

import numpy as np
import ml_dtypes

import concourse.bass as bass
import concourse.mybir as mybir
from concourse import tile as _tile
from concourse.tile import TileContext
from concourse.bass_utils import run_bass_kernel_spmd

dt = mybir.dt

B, F, T, K = 16, 256, 2048, 20
NCORES = 8
ROWS = B // NCORES          # batch rows per core
NBLK = T // 128             # t-blocks per batch row
FCH = F // 128              # f chunks (contraction is 2 x 128 partitions)
CB = 128                    # c-sims block width per strip
WMAX = CB + T               # widest strip (tau=0)
SCALE = 63.0                # int8 code per logit unit; logits in [-2, 2]

# ---------------------------------------------------------------------------
# Walrus in this container rejects instructions that carry more than one
# semaphore wait ("Too many sync wait commands").  Two shims fix that: the
# tile tail drain gets its waits on single-wait NOPs, and a post-pass splits
# any remaining multi-wait instruction.
# ---------------------------------------------------------------------------


def _patched_drain_and_barrier(self, tick_clock, wait_clock):
    nop0 = self.nc.sync.nop(nofuse=True, hint="tail_wait")
    wait_clock.add_sem_waits(
        nop0.ins, _tile.ScopedClock({None: tick_clock.global_clock})
    )
    si = nop0.ins.sync_info
    if si is not None and len(si.on_wait) > 1:
        waits = list(si.on_wait)
        nop0.ins.sync_info = mybir.SyncInfo(
            on_wait=waits[:1], on_update=list(si.on_update)
        )
        for w in waits[1:]:
            nopi = self.nc.sync.nop(nofuse=True, hint="tail_wait")
            nopi.ins.sync_info = mybir.SyncInfo(on_wait=[w], on_update=[])
    self.nc.sync.drain()
    self.nc.all_engine_barrier()
    assert self.sems is not None
    popped = self.nc._tile_sem_poison_stack.pop()
    assert popped is self._sem_poison
    self.nc.clear_and_free_semaphores(list(self.sems.allocated().values()))
    self.nc.all_engine_barrier()


_tile.TileContext._drain_and_barrier = _patched_drain_and_barrier

_wnop_counter = [0]


def split_excess_waits(nc, cap=1):
    for f in nc.m.functions:
        for bb in f.blocks:
            insts = bb.instructions
            out = []
            changed = False
            for inst in list(insts):
                si = getattr(inst, "sync_info", None)
                waits = list(si.on_wait) if si is not None else []
                if len(waits) > cap:
                    keep = waits[-cap:]
                    for w in waits[: len(waits) - cap]:
                        _wnop_counter[0] += 1
                        nop = mybir.InstNoOp(
                            name=f"wnop-{_wnop_counter[0]}", ins=[], outs=[]
                        )
                        nop.engine = inst.engine
                        nop.sync_info = mybir.SyncInfo(on_wait=[w], on_update=[])
                        out.append(nop)
                    inst.sync_info = mybir.SyncInfo(
                        on_wait=keep, on_update=list(si.on_update)
                    )
                    changed = True
                out.append(inst)
            if changed:
                insts[:] = out


def dedup_ldweights(nc):
    """The tile lowering emits an explicit InstLdweights before every
    InstMatmult.  Consecutive matmuls that share the stationary operand don't
    need the reload -- the PE keeps its weights."""
    n = 0
    for f in nc.m.functions:
        for bb in f.blocks:
            insts = bb.instructions
            last_key = None
            out = []
            changed = False
            for inst in list(insts):
                tn = type(inst).__name__
                if tn == "InstLdweights":
                    key = (
                        str(inst.ins[0]),
                        tuple(inst.tile_position or ()),
                        tuple(inst.tile_size or ()),
                        bool(inst.is_transpose),
                    )
                    if key == last_key:
                        nop = mybir.InstNoOp(name=f"ldwnop-{n}", ins=[], outs=[])
                        n += 1
                        nop.engine = inst.engine
                        si = inst.sync_info
                        if si is not None:
                            nop.sync_info = mybir.SyncInfo(
                                on_wait=list(si.on_wait), on_update=list(si.on_update)
                            )
                        out.append(nop)
                        changed = True
                        continue
                    last_key = key
                elif tn == "InstMatmult":
                    if inst.is_transpose:
                        last_key = None
                out.append(inst)
            if changed:
                insts[:] = out
    return n


# ---------------------------------------------------------------------------
# Device program
# ---------------------------------------------------------------------------


def build_program():
    nc = bass.Bass("TRN2", num_devices=NCORES)
    zs_in = nc.dram_tensor("zs", [ROWS, F, T], dt.bfloat16, kind="ExternalInput")
    cs_in = nc.dram_tensor("cs", [ROWS, F, T], dt.bfloat16, kind="ExternalInput")
    tri_out = nc.dram_tensor("tri", [ROWS, T, T], dt.int8, kind="ExternalOutput")
    aux_out = nc.dram_tensor("aux", [ROWS, T], dt.float32, kind="ExternalOutput")

    with TileContext(nc) as tc:
        with (
            tc.tile_pool(name="io", bufs=1) as io_pool,
            tc.tile_pool(name="work", bufs=2) as work,
            tc.tile_pool(name="outp", bufs=4) as outp,
            tc.tile_pool(name="gram_ps", bufs=4, space="PSUM") as gram_ps,
        ):
            # all loads up front; distinct tags so nothing rotates.
            # z before c (strips need z first; the c-block is last in each
            # pass), and row-0 z in halves so the PE can start sooner.
            zs16 = {}
            cs16 = {}
            for r in range(ROWS):
                for j in range(FCH):
                    zs16[r, j] = io_pool.tile(
                        [128, T], dt.bfloat16, name=f"zs_r{r}j{j}", tag=f"zs_r{r}j{j}"
                    )
                    cs16[r, j] = io_pool.tile(
                        [128, T], dt.bfloat16, name=f"cs_r{r}j{j}", tag=f"cs_r{r}j{j}"
                    )
            # Strips run in order [15..8, 0..7]: the first eight need only the
            # SECOND half of z.  The SP queue carries ONLY those two DMAs so
            # they get full HBM bandwidth; every other load is issued from the
            # ACT queue, gated in program order behind early strip copies:
            #   si==0 -> z-h0,  si==2 -> c row 0,  si==5 -> z row 1,
            #   si==8 -> c row 1.
            H = T // 2
            for j in range(FCH):
                nc.sync.dma_start(
                    out=zs16[0, j][:, H:T], in_=zs_in[0, 128 * j : 128 * (j + 1), H:T]
                )

            ones16 = io_pool.tile([128, 128], dt.bfloat16, name="ones16", tag="ones")
            nc.vector.memset(ones16[:], 1.0)

            # greedy cost-balanced copy assignment (ACT: ~(172+FD)/1.2GHz,
            # DVE: ~(120+FD)/0.96GHz per piece)
            eng_load = [0.0, 0.0]   # ns accumulated: [ACT, DVE]

            def emit_copy(dst, src, fd, force=None):
                cost = ((172 + fd) / 1.2, (120 + fd) / 0.96)
                if force is None:
                    e = 0 if eng_load[0] + cost[0] <= eng_load[1] + cost[1] else 1
                else:
                    e = force
                eng_load[e] += cost[e]
                if e == 0:
                    nc.scalar.copy(dst, src)
                else:
                    nc.vector.tensor_copy(dst, src)

            def emit_strip(r, tau, force_copy=None):
                t0 = 128 * tau
                W = T - t0                      # z-sims columns
                otile = outp.tile([128, T], dt.int8, name="otile", tag="otile")
                # psum pieces of <=1024 cols
                pieces = [(po, min(1024, W - po)) for po in range(0, W, 1024)]
                ps_tiles = []
                for pi, (po, pw) in enumerate(pieces):
                    ps = gram_ps.tile([128, 1024], dt.float32, name=f"ps{pi}", tag="g")
                    ps_tiles.append(ps)
                for j in range(FCH):
                    lhsT = zs16[r, j][:, t0 : t0 + 128]
                    st = j == 0
                    sp = j == FCH - 1
                    for (po, pw), ps in zip(pieces, ps_tiles):
                        for col in range(0, pw, 512):
                            w = min(512, pw - col)
                            u0 = t0 + po + col
                            nc.tensor.matmul(
                                ps[:, col : col + w],
                                lhsT,
                                zs16[r, j][:, u0 : u0 + w],
                                start=st,
                                stop=sp,
                            )
                for (po, pw), ps in zip(pieces, ps_tiles):
                    emit_copy(otile[:, po : po + pw], ps[:, :pw], pw, force=force_copy)
                nc.sync.dma_start(out=tri_out[r, t0 : t0 + 128, 0:W], in_=otile[:, :W])

            def emit_pos(r):
                # positives: pos[t] = sum_f zs[f,t]*cs[f,t].  DVE does the
                # elementwise product; a ones-stationary matmul reduces over
                # partitions (result replicated on every partition); partition
                # 0 ships as exact fp32.
                prods = []
                for j in range(FCH):
                    pr = work.tile([128, T], dt.bfloat16, name=f"prod{j}", tag=f"prod{j}")
                    nc.vector.tensor_tensor(
                        out=pr[:], in0=zs16[r, j][:], in1=cs16[r, j][:],
                        op=mybir.AluOpType.mult,
                    )
                    prods.append(pr)
                pos_sb = work.tile([1, T], dt.float32, name="pos_sb", tag="pos_sb")
                for half in range(2):
                    ps = gram_ps.tile([128, 1024], dt.float32, name="pps", tag="g")
                    for cchunk in range(2):
                        sl = slice(1024 * half + 512 * cchunk, 1024 * half + 512 * (cchunk + 1))
                        for j in range(FCH):
                            nc.tensor.matmul(
                                ps[:, 512 * cchunk : 512 * (cchunk + 1)],
                                ones16[:], prods[j][:, sl],
                                start=(j == 0), stop=(j == FCH - 1),
                            )
                    emit_copy(
                        pos_sb[0:1, 1024 * half : 1024 * (half + 1)], ps[0:1, :], 1024
                    )
                nc.sync.dma_start(out=aux_out[r : r + 1, :], in_=pos_sb[:])

            strip_order = list(range(NBLK - 1, NBLK // 2 - 1, -1)) + list(
                range(NBLK // 2)
            )
            for r in range(ROWS):
                sid = nc.enter_named_scope(f"gram_r{r}", False)[0]
                for si, tau in enumerate(strip_order):
                    # the very first strip's copies are forced onto ACT so the
                    # gated loads below are issued right after it executes
                    emit_strip(r, tau, force_copy=0 if (r == 0 and si == 0) else None)
                    if r == 0 and si == 0:
                        for j in range(FCH):
                            nc.scalar.dma_start(
                                out=zs16[0, j][:, 0:H],
                                in_=zs_in[0, 128 * j : 128 * (j + 1), 0:H],
                            )
                    if r == 0 and si == 2:
                        for j in range(FCH):
                            nc.scalar.dma_start(
                                out=cs16[0, j][:],
                                in_=cs_in[0, 128 * j : 128 * (j + 1), :],
                            )
                    if r == 0 and si == 5:
                        for rr in range(1, ROWS):
                            for j in range(FCH):
                                nc.scalar.dma_start(
                                    out=zs16[rr, j][:],
                                    in_=zs_in[rr, 128 * j : 128 * (j + 1), :],
                                )
                    if r == 0 and si == 8:
                        for rr in range(1, ROWS):
                            for j in range(FCH):
                                nc.scalar.dma_start(
                                    out=cs16[rr, j][:],
                                    in_=cs_in[rr, 128 * j : 128 * (j + 1), :],
                                )
                    if si == 13:
                        emit_pos(r)
                nc.leave_named_scope(f"gram_r{r}", sid, False)

    dedup_ldweights(nc)
    split_excess_waits(nc)
    return nc


_PROGRAM = None


def _get_program():
    global _PROGRAM
    if _PROGRAM is None:
        _PROGRAM = build_program()
    return _PROGRAM


SQRT2 = np.float32(np.sqrt(2.0))


def kernel(z, c, negative_inds, _trace=False):
    z = np.asarray(z, dtype=np.float32)
    c = np.asarray(c, dtype=np.float32)
    ni = np.asarray(negative_inds)
    assert z.shape == (B, F, T) and c.shape == (B, F, T + 1)

    # host prep: per-column normalize (folds cosine denominators and 1/TEMP),
    # pre-scale by sqrt(SCALE) so the device Gram is already int8-coded.
    s = np.float32(np.sqrt(SCALE))
    zn = np.sqrt(np.einsum("bft,bft->bt", z, z, dtype=np.float32))
    zsf = z * (s * SQRT2 / zn)[:, None, :]
    zs16 = zsf.astype(ml_dtypes.bfloat16)
    csl = np.ascontiguousarray(c[:, :, 1:])
    cn = np.sqrt(np.einsum("bft,bft->bt", csl, csl, dtype=np.float32))
    csf = csl * (s * SQRT2 / cn)[:, None, :]
    cs16 = csf.astype(ml_dtypes.bfloat16)

    nc = _get_program()
    in_maps = []
    for core in range(NCORES):
        rs = slice(core * ROWS, (core + 1) * ROWS)
        in_maps.append({"zs": zs16[rs], "cs": cs16[rs]})

    res = run_bass_kernel_spmd(nc, in_maps, list(range(NCORES)), trace=_trace)

    S = np.concatenate(
        [res.results[i]["tri"].reshape(ROWS, T, T) for i in range(NCORES)], axis=0
    )  # [B, T, T] int8 (left-aligned upper-triangle strips)
    pos = np.concatenate(
        [res.results[i]["aux"].reshape(ROWS, T) for i in range(NCORES)], axis=0
    )  # [B, T] fp32, already SCALE * logit

    # host-side pure indexing + int8 decode
    tarr = np.arange(T)
    tau_t = tarr >> 7                                   # [T]
    n = ni.reshape(B, T, K).astype(np.int64)            # values in [0, T-2]
    up = n >= (tau_t << 7)[None, :, None]               # u in own strip?
    t_b = np.broadcast_to(tarr[None, :, None], n.shape)
    rowsel = np.where(up, t_b, n)
    colsel = np.where(
        up,
        n - (tau_t << 7)[None, :, None],
        t_b - ((n >> 7) << 7),
    )
    bidx = np.arange(B)[:, None, None]
    neg_i8 = S[bidx, rowsel, colsel]                    # [B, T, K]

    inv = np.float32(1.0 / SCALE)
    logits = np.empty((B, T, K + 1), dtype=np.float32)
    logits[:, :, 0] = pos * inv
    logits[:, :, 1:] = neg_i8.astype(np.float32) * inv
    out = logits.reshape(B * T, K + 1)
    if _trace:
        return out, res
    return out


if __name__ == "__main__":
    rng = np.random.default_rng(0)
    z = rng.standard_normal((B, F, T), dtype=np.float32)
    c = rng.standard_normal((B, F, T + 1), dtype=np.float32)
    ni = rng.integers(0, T - 1, size=(B, T * K)).astype(np.int64)
    out = kernel(z=z, c=c, negative_inds=ni)
    print("out", out.shape, out.dtype, np.isfinite(out).all())
